# revision 48
# baseline (speedup 1.0000x reference)
"""Causal single-head attention (B=4, T=4096, D=1024, H=64) on 8 TRN2 cores.

Sharding: 2 cores per batch; queries split for causal load balance:
  half0 (cores 0-3):  query tiles {0,3,4,7} (x512 rows), keys [0,4096)
  half1 (cores 4-7):  query tiles {1,2,5,6},              keys [0,3584)
Both halves own 72 key chunks of attention work.

The host pre-transposes and bf16-casts x per core (x^T, group-major so
every (group, partition) DMA line is one contiguous 8KB run).  Weights
are host-packed into SBUF partition layout: [Wq] and [Wk|Wv].
HBM traffic per core: 8.4MB bf16 instead of 16.8MB f32.

One fully-specialized If/Else (engines PE/Act/DVE); everything except
DMAs runs inside the branch.  Query projections (m=64 Wq pass) are
emitted early and independently of the causal key order, so each tile's
fill chunks run as soon as their keys project; only diagonal chunks
wait for their own group.  Attention segments are merged into pipelined
runs (scores of pair i+1 emitted during exp(i) on Act; the next
projection serves as PE filler under the last exp of a run).

Per key group g: pkv[128,512] = [Wk|Wv]^T x^T (single m=128 pass); the
kT row dup for row-packed score pairs comes from partition-shifted
Act/DVE copies straight out of PSUM.  PV accumulates vsb chunks
[v | ones] (m=128) so the softmax denominator lands replicated on psum
rows 64:128: epilogue = partition-shifted DVE reciprocal + aligned
multiply.  Output stays transposed [64, 2048]; the host transposes
back during unsharding.
"""

import numpy as np
import ml_dtypes

import concourse.bass as bass
import concourse.mybir as mybir
from concourse import bacc
from concourse.tile import TileContext
from concourse.masks import make_identity
from concourse.bass_utils import run_bass_kernel_spmd

B, T, D, H = 4, 4096, 1024, 64
NCORES = 8
NQ = 2048
SCALE = 1.0 / np.sqrt(D)  # 1/32
BF16 = ml_dtypes.bfloat16

QTILES = {0: [0, 3, 4, 7], 1: [1, 2, 5, 6]}
XORDER = [0, 1, 3, 2, 4, 5, 6, 7]

# event schedules: ("qp", tile) = query projection, ("kv", group) = key/value
# projection, ("A", tile, plo, phi) = attention pairs for chunk positions
# [plo, phi) of that tile (chunk c is ready once kv(c//4) ran; diagonal
# chunks 4t..4t+3 are in group t; tile t's n = 4t+4)
SCHED = {
    0: [("qp", 0), ("kv", 0), ("A", 0, 0, 4),
        ("qp", 3), ("kv", 1), ("A", 3, 0, 8),
        ("kv", 2), ("A", 3, 8, 12),
        ("kv", 3), ("A", 3, 12, 16),
        ("qp", 4), ("A", 4, 0, 16),
        ("kv", 4), ("A", 4, 16, 20),
        ("qp", 7), ("kv", 5), ("A", 7, 0, 24),
        ("kv", 6), ("A", 7, 24, 28),
        ("kv", 7), ("A", 7, 28, 32)],
    1: [("kv", 0), ("qp", 1), ("kv", 1), ("A", 1, 0, 8),
        ("qp", 2), ("kv", 2), ("A", 2, 0, 12),
        ("qp", 5), ("kv", 3), ("A", 5, 0, 16),
        ("kv", 4), ("A", 5, 16, 20),
        ("qp", 6), ("kv", 5), ("A", 5, 20, 24), ("A", 6, 0, 24),
        ("kv", 6), ("A", 6, 24, 28)],
}

_CACHE = {}


def _build():
    if "nc" in _CACHE:
        return _CACHE["nc"]
    f32 = mybir.dt.float32
    bf16 = mybir.dt.bfloat16
    AF = mybir.ActivationFunctionType

    nc = bacc.Bacc(None, target_bir_lowering=False)
    # x^T in group-major layout: x_d[g, p, c*512+t'] = x[512g+t', 128c+p]
    x_d = nc.declare_dram_parameter("xt", [8, 128, 4096], bf16, isOutput=False)
    wq_d = nc.declare_dram_parameter("wq", [128, 512], bf16, isOutput=False)
    wkv_d = nc.declare_dram_parameter("wkv", [128, 1024], bf16, isOutput=False)
    out_d = nc.declare_dram_parameter("out", [H, NQ], f32, isOutput=True)

    with TileContext(nc) as tc:
        with (
            tc.tile_pool(name="persist", bufs=1) as pp,
            tc.tile_pool(name="work", bufs=2) as pw,
        ):
            # ---- weights (sync) + x^T groups ----
            wq = pp.tile([128, 512], bf16, tag="wq")
            nc.sync.dma_start(out=wq[:, :], in_=wq_d[:, :])
            wkv = pp.tile([128, 1024], bf16, tag="wkv")
            nc.sync.dma_start(out=wkv[:, :], in_=wkv_d[:, :])

            xsb = pp.tile([128, 8 * T], bf16, tag="xsb")  # [p, (dc, t)]
            xview = xsb[:, :].rearrange("p (c t) -> p c t", t=T)
            for i, g in enumerate(XORDER):
                eng = nc.gpsimd if i < 4 else nc.sync
                if i == 0:  # split so the first projection starts sooner
                    for hh in range(2):
                        eng.dma_start(
                            out=xview[:, 4 * hh: 4 * (hh + 1), 512 * g: 512 * (g + 1)],
                            in_=x_d[g, :, :].rearrange(
                                "p (c t) -> p c t", t=512)[:, 4 * hh: 4 * (hh + 1), :])
                else:
                    eng.dma_start(
                        out=xview[:, :, 512 * g: 512 * (g + 1)],
                        in_=x_d[g, :, :].rearrange("p (c t) -> p c t", t=512))

            # ---- constants ----
            ident_f = pp.tile([128, 128], f32, tag="idf")
            make_identity(nc, ident_f[:, :])
            ident_b = pp.tile([128, 128], bf16, tag="idb")
            nc.vector.tensor_copy(ident_b[:, :], ident_f[:, :])

            # mask_big[p, g] = 1 iff g >= p + 384 (else 0)
            mask_f = pp.tile([128, 896], f32, tag="mkf")
            nc.gpsimd.memset(mask_f[:, :], 0.0)
            nc.gpsimd.affine_select(
                out=mask_f[:, :], in_=mask_f[:, :],
                compare_op=mybir.AluOpType.is_gt, fill=1.0,
                base=384, pattern=[[-1, 896]], channel_multiplier=1,
            )
            mask_b = pp.tile([128, 896], bf16, tag="mkb")
            nc.vector.tensor_copy(mask_b[:, :], mask_f[:, :])

            # persistent activations
            kT = pp.tile([128, T], bf16, tag="kT")     # k^T; rows 64:128 dup
            qT = pp.tile([128, T], bf16, tag="qT")     # q^T; rows 0:64 dup
            # vsb chunk ch: cols 0:64 = v rows of key chunk ch, cols 64:128 = 1
            # -> PV matmul (m=128) yields numerator on psum rows 0:64 and the
            #    denominator replicated on rows 64:128 (same cycle count)
            vsb = pp.tile([128, 32 * 128], bf16, tag="vsb")
            nc.gpsimd.memset(vsb[:, :], 1.0)

            # preload the exp activation table early (hide the ~1.3us load)
            warm = pw.tile([1, 1], f32, tag="warm")
            nc.scalar.activation(warm[:, :], mask_b[0:1, 0:1], AF.Exp, scale=1.0)

            with (
                tc.tile_pool(name="psA", bufs=1, space="PSUM") as psA,
                tc.tile_pool(name="ps2", bufs=1, space="PSUM") as ps2,
                tc.tile_pool(name="ps3", bufs=1, space="PSUM") as ps3,
            ):
                # everything pre-allocated OUTSIDE the If (pool allocation
                # inside conditional branches breaks Tile's wait assignment)
                pjA = psA.tile([128, 512], f32, tag="pjA")   # [k|v] pass
                pjB = psA.tile([128, 512], f32, tag="pjB")   # vn transpose out
                ps_bufs = [ps2.tile([128, 1024], f32, tag=f"sc{i}", name=f"scb{i}") for i in range(2)]
                pT_bufs = [pw.tile([128, 1024], bf16, tag=f"pT{i}", name=f"pTb{i}") for i in range(3)]
                po_bufs = [ps3.tile([128, 512], f32, tag=f"po{i}", name=f"pob{i}") for i in range(2)]
                rc_bufs = [pw.tile([128, 512], f32, tag=f"rc{i}", name=f"rcb{i}") for i in range(2)]
                vT_bufs = [pw.tile([64, 512], bf16, tag=f"vT{i}", name=f"vTb{i}") for i in range(2)]
                outsb = pw.tile([64, 2048], f32, tag="outsb")

                def xrhs(dc, g):
                    return xsb[:, 4096 * dc + 512 * g: 4096 * dc + 512 * (g + 1)]

                def body(half):
                    qtiles = QTILES[half]
                    slot_of = {t: i for i, t in enumerate(qtiles)}
                    # pending epilogue pieces per po bank: the 3.4us DVE
                    # reciprocal would block the in-order DVE queue (and the
                    # kT dups the next scores need), so it is split into
                    # 128-col pieces dribbled between later pairs
                    state = {"gi": 0, "pend": {0: [], 1: []}}

                    def dribble(k):
                        for bank in (0, 1):
                            pend = state["pend"][bank]
                            while k > 0 and pend:
                                pend.pop(0)()
                                k -= 1

                    def drain(bank):
                        pend = state["pend"][bank]
                        while pend:
                            pend.pop(0)()

                    def emit_qp(t, t2=None):
                        # query projection m=64, lands in the scores psum
                        # rotation (consumes one gi slot); a second tile
                        # col-packs into psum rows 64:128 and runs
                        # concurrently on the PE array
                        gi = state["gi"]
                        state["gi"] = gi + 1
                        qp = ps_bufs[gi % 2]
                        tsl = slice(512 * t, 512 * (t + 1))
                        for dc in range(8):
                            nc.tensor.matmul(
                                qp[0:64, 0:512],
                                lhsT=wq[:, 64 * dc: 64 * (dc + 1)],
                                rhs=xrhs(dc, t), start=(dc == 0), stop=(dc == 7))
                            if t2 is not None:
                                nc.tensor.matmul(
                                    qp[64:128, 0:512],
                                    lhsT=wq[:, 64 * dc: 64 * (dc + 1)],
                                    rhs=xrhs(dc, t2), start=(dc == 0), stop=(dc == 7))
                        nc.scalar.copy(qT[0:64, tsl], qp[0:64, 0:512])
                        nc.vector.tensor_copy(qT[64:128, tsl], qp[0:64, 0:512])
                        if t2 is not None:
                            tsl2 = slice(512 * t2, 512 * (t2 + 1))
                            nc.vector.tensor_copy(qT[64:128, tsl2], qp[64:128, 0:512])
                            nc.scalar.copy(qT[0:64, tsl2], qp[64:128, 0:512])

                    def emit_kv(g):
                        tsl = slice(512 * g, 512 * (g + 1))
                        for dc in range(8):
                            nc.tensor.matmul(
                                pjA[:, :], lhsT=wkv[:, 128 * dc: 128 * (dc + 1)],
                                rhs=xrhs(dc, g), start=(dc == 0), stop=(dc == 7))
                        nc.scalar.copy(kT[0:64, tsl], pjA[0:64, :])
                        nc.vector.tensor_copy(kT[64:128, tsl], pjA[0:64, :])
                        vT = vT_bufs[g % 2]
                        nc.vector.tensor_copy(vT[:, :], pjA[64:128, :])
                        for c in range(4):
                            nc.tensor.matmul(
                                pjB[:, 64 * c: 64 * (c + 1)],
                                lhsT=vT[0:64, 128 * c: 128 * (c + 1)],
                                rhs=ident_b[0:64, 0:64], start=True, stop=True)
                        nc.vector.tensor_copy(
                            vsb[:, 512 * g: 512 * (g + 1)].rearrange(
                                "p (c h) -> p c h", h=128)[:, :, 0:64],
                            pjB[:, 0:256].rearrange("p (c h) -> p c h", h=64))

                    def emit_event(ev):
                        if ev[0] == "qp":
                            emit_qp(ev[1])
                        elif ev[0] == "qp2":
                            emit_qp(ev[1], ev[2])
                        else:
                            emit_kv(ev[1])

                    def emit_attn_run(run, filler):
                        gi0 = state["gi"]
                        pairs = []
                        for _, t, plo, phi in run:
                            if plo == 0:  # first PVs on this po bank: the
                                # prior user's epilogue must be fully emitted
                                drain(slot_of[t] % 2)
                            for p in range(plo, phi, 2):
                                pairs.append((t, slot_of[t], p, 4 * t + 4))

                        def emit_scores(j):
                            t, slot, p, n = pairs[j]
                            tsl = slice(512 * t, 512 * (t + 1))
                            ps = ps_bufs[(gi0 + j) % 2]
                            nc.tensor.matmul(
                                ps[:, 0:512],
                                lhsT=kT[0:64, 128 * p: 128 * (p + 1)],
                                rhs=qT[0:64, tsl], start=True, stop=True)
                            nc.tensor.matmul(
                                ps[:, 512:1024],
                                lhsT=kT[64:128, 128 * (p + 1): 128 * (p + 2)],
                                rhs=qT[64:128, tsl], start=True, stop=True)

                        emit_scores(0)
                        for j, (t, slot, p, n) in enumerate(pairs):
                            ps = ps_bufs[(gi0 + j) % 2]
                            pT = pT_bufs[(gi0 + j) % 3]
                            po = po_bufs[slot % 2]
                            nc.scalar.activation(pT[:, :], ps[:, :], AF.Exp, scale=SCALE)
                            # keep PE busy during exp(j)
                            if j + 1 < len(pairs):
                                emit_scores(j + 1)
                            elif filler is not None:
                                emit_event(filler)
                            for jj in range(2):
                                pp_ = p + jj
                                if pp_ >= n - 4:  # diagonal chunk: causal mask
                                    delta = 128 * (pp_ - (n - 4))
                                    nc.vector.tensor_mul(
                                        pT[:, 512 * jj: 512 * (jj + 1)],
                                        pT[:, 512 * jj: 512 * (jj + 1)],
                                        mask_b[:, 384 - delta: 896 - delta])
                                nc.tensor.matmul(
                                    po[:, :],
                                    lhsT=vsb[:, 128 * pp_: 128 * (pp_ + 1)],
                                    rhs=pT[:, 512 * jj: 512 * (jj + 1)],
                                    start=(pp_ == 0), stop=(pp_ == n - 1))
                            if p + 2 == n:
                                # epilogue pieces: shifted reciprocal of the
                                # replicated denominator + aligned multiply
                                rc = rc_bufs[slot % 2]

                                def piece(q, slot=slot, po=po, rc=rc):
                                    cs = slice(128 * q, 128 * (q + 1))
                                    os = slice(512 * slot + 128 * q,
                                               512 * slot + 128 * (q + 1))
                                    nc.vector.reciprocal(
                                        rc[0:64, cs], po[64:128, cs])
                                    nc.vector.tensor_mul(
                                        outsb[:, os], po[0:64, cs], rc[0:64, cs])

                                state["pend"][slot % 2].extend(
                                    (lambda q=q: piece(q)) for q in range(4))
                            else:
                                dribble(1)
                        state["gi"] = gi0 + len(pairs)

                    events = SCHED[half]
                    i = 0
                    while i < len(events):
                        if events[i][0] != "A":
                            emit_event(events[i])
                            dribble(1)
                            i += 1
                            continue
                        run = []
                        while i < len(events) and events[i][0] == "A":
                            run.append(events[i])
                            i += 1
                        filler = None
                        if i < len(events):
                            filler = events[i]
                            i += 1
                        emit_attn_run(run, filler)
                    drain(0)
                    drain(1)

                pid = nc.partition_id(engines=[
                    mybir.EngineType.PE, mybir.EngineType.Activation,
                    mybir.EngineType.DVE])
                with tc.If(pid < 4) as cmp:
                    body(0)
                with cmp.Else():
                    body(1)

                # store after the If, sliced per slot so early slots overlap;
                # the last slot goes out in quarters as its epilogue pieces
                # complete
                for slot in range(3):
                    nc.sync.dma_start(
                        out=out_d[:, 512 * slot: 512 * (slot + 1)],
                        in_=outsb[:, 512 * slot: 512 * (slot + 1)])
                for q in range(4):
                    qs = slice(512 * 3 + 128 * q, 512 * 3 + 128 * (q + 1))
                    nc.sync.dma_start(out=out_d[:, qs], in_=outsb[:, qs])

    nc.compile()
    _CACHE["nc"] = nc
    return nc


def _in_maps(x, Wq, Wk, Wv):
    def pack128(w):  # [1024, m] -> [128, 8*m] partition layout
        m = w.shape[1]
        return np.ascontiguousarray(
            w.astype(BF16).reshape(8, 128, m).transpose(1, 0, 2).reshape(128, 8 * m))

    Wk, Wq, Wv = (np.asarray(a) for a in (Wk, Wq, Wv))
    wq_sb = pack128(Wq)
    wkv_sb = pack128(np.concatenate([Wk, Wv], axis=1))
    # X5[g, p, c, t'] = x[512g+t', 128c+p]  (8KB contiguous per (g, p))
    xts = [np.ascontiguousarray(
        np.asarray(x[b]).astype(BF16).reshape(8, 512, 8, 128)
        .transpose(0, 3, 2, 1).reshape(8, 128, 4096)) for b in range(B)]
    maps = []
    for c in range(NCORES):
        b = c % 4
        maps.append({"xt": xts[b], "wq": wq_sb, "wkv": wkv_sb})
    return maps


def _install_profile_shim():
    import sys, types
    import concourse.bass_utils as bu
    bu.upload_artifacts = lambda tmpdir: "local://" + tmpdir
    if "antenv.axon_hooks" in sys.modules:
        return
    mod = types.ModuleType("antenv.axon_hooks")
    holder = []
    mod.set_axon_ntff_profile_hook = holder.append
    mod.get_axon_ntff_profile_hook = lambda: holder[-1] if holder else None
    sys.modules["antenv.axon_hooks"] = mod
    import antenv
    antenv.axon_hooks = mod
    from trn_agent_boot.trn_boot import _ntff_profile_via_ctypes
    mod.set_axon_ntff_profile_hook(_ntff_profile_via_ctypes("/opt/axon/libaxon_pjrt.so"))


def kernel(x, Wq, Wk, Wv, _want_profile=False):
    if _want_profile:
        _install_profile_shim()
    nc = _build()
    maps = _in_maps(x, Wq, Wk, Wv)
    res = run_bass_kernel_spmd(nc, maps, core_ids=list(range(NCORES)),
                               trace=_want_profile)
    out = np.empty((B, T, H), np.float32)
    for c in range(NCORES):
        b, half = c % 4, c // 4
        r = np.asarray(res.results[c]["out"])  # [64, 2048]
        for slot, t in enumerate(QTILES[half]):
            out[b, 512 * t: 512 * (t + 1)] = r[:, 512 * slot: 512 * (slot + 1)].T
    if _want_profile:
        return out, res
    return out


# revision 49
# speedup vs baseline: 1.0417x; 1.0417x over previous
"""Causal single-head attention (B=4, T=4096, D=1024, H=64) on 8 TRN2 cores.

Sharding: 2 cores per batch; queries split for causal load balance:
  half0 (cores 0-3):  query tiles {0,3,4,7} (x512 rows), keys [0,4096)
  half1 (cores 4-7):  query tiles {1,2,5,6},              keys [0,3584)
Both halves own 72 key chunks of attention work.

The host pre-transposes and bf16-casts x per core (x^T, group-major so
every (group, partition) DMA line is one contiguous 8KB run).  Weights
are host-packed into SBUF partition layout: [Wq] and [Wk|Wv].
HBM traffic per core: 8.4MB bf16 instead of 16.8MB f32.

One fully-specialized If/Else (engines PE/Act/DVE); everything except
DMAs runs inside the branch.  Query projections (m=64 Wq pass) are
emitted early and independently of the causal key order, so each tile's
fill chunks run as soon as their keys project; only diagonal chunks
wait for their own group.  Attention segments are merged into pipelined
runs (scores of pair i+1 emitted during exp(i) on Act; the next
projection serves as PE filler under the last exp of a run).

Per key group g: pkv[128,512] = [Wk|Wv]^T x^T (single m=128 pass); the
kT row dup for row-packed score pairs comes from partition-shifted
Act/DVE copies straight out of PSUM.  PV accumulates vsb chunks
[v | ones] (m=128) so the softmax denominator lands replicated on psum
rows 64:128: epilogue = partition-shifted DVE reciprocal + aligned
multiply.  Output stays transposed [64, 2048]; the host transposes
back during unsharding.
"""

import numpy as np
import ml_dtypes

import concourse.bass as bass
import concourse.mybir as mybir
from concourse import bacc
from concourse.tile import TileContext
from concourse.masks import make_identity
from concourse.bass_utils import run_bass_kernel_spmd

B, T, D, H = 4, 4096, 1024, 64
NCORES = 8
NQ = 2048
SCALE = 1.0 / np.sqrt(D)  # 1/32
BF16 = ml_dtypes.bfloat16

QTILES = {0: [0, 3, 4, 7], 1: [1, 2, 5, 6]}
XORDER = [0, 1, 3, 2, 4, 5, 6, 7]

# event schedules: ("qp", tile) = query projection, ("kv", group) = key/value
# projection, ("A", tile, plo, phi) = attention pairs for chunk positions
# [plo, phi) of that tile (chunk c is ready once kv(c//4) ran; diagonal
# chunks 4t..4t+3 are in group t; tile t's n = 4t+4)
SCHED = {
    0: [("qp", 0), ("kv", 0), ("A", 0, 0, 4),
        ("qp", 3), ("kv", 1), ("A", 3, 0, 8),
        ("kv", 2), ("A", 3, 8, 12),
        ("kv", 3), ("A", 3, 12, 16),
        ("qp", 4), ("A", 4, 0, 16),
        ("kv", 4), ("A", 4, 16, 20),
        ("qp", 7), ("kv", 5), ("A", 7, 0, 24),
        ("kv", 6), ("A", 7, 24, 28),
        ("kv", 7), ("A", 7, 28, 32)],
    1: [("kv", 0), ("qp", 1), ("kv", 1), ("A", 1, 0, 8),
        ("qp", 2), ("kv", 2), ("A", 2, 0, 12),
        ("qp", 5), ("kv", 3), ("A", 5, 0, 16),
        ("kv", 4), ("A", 5, 16, 20),
        ("qp", 6), ("kv", 5), ("A", 5, 20, 24), ("A", 6, 0, 24),
        ("kv", 6), ("A", 6, 24, 28)],
}

_CACHE = {}


def _build():
    if "nc" in _CACHE:
        return _CACHE["nc"]
    f32 = mybir.dt.float32
    bf16 = mybir.dt.bfloat16
    AF = mybir.ActivationFunctionType

    nc = bacc.Bacc(None, target_bir_lowering=False)
    # x^T in group-major layout: x_d[g, p, c*512+t'] = x[512g+t', 128c+p]
    x_d = nc.declare_dram_parameter("xt", [8, 128, 4096], bf16, isOutput=False)
    wq_d = nc.declare_dram_parameter("wq", [128, 512], bf16, isOutput=False)
    wkv_d = nc.declare_dram_parameter("wkv", [128, 1024], bf16, isOutput=False)
    out_d = nc.declare_dram_parameter("out", [H, NQ], f32, isOutput=True)

    with TileContext(nc) as tc:
        with (
            tc.tile_pool(name="persist", bufs=1) as pp,
            tc.tile_pool(name="work", bufs=2) as pw,
        ):
            # ---- weights (sync) + x^T groups ----
            wq = pp.tile([128, 512], bf16, tag="wq")
            nc.sync.dma_start(out=wq[:, :], in_=wq_d[:, :])
            wkv = pp.tile([128, 1024], bf16, tag="wkv")
            nc.sync.dma_start(out=wkv[:, :], in_=wkv_d[:, :])

            xsb = pp.tile([128, 8 * T], bf16, tag="xsb")  # [p, (dc, t)]
            xview = xsb[:, :].rearrange("p (c t) -> p c t", t=T)
            for i, g in enumerate(XORDER):
                eng = nc.gpsimd if i < 4 else nc.sync
                if i == 0:  # split so the first projection starts sooner
                    for hh in range(2):
                        eng.dma_start(
                            out=xview[:, 4 * hh: 4 * (hh + 1), 512 * g: 512 * (g + 1)],
                            in_=x_d[g, :, :].rearrange(
                                "p (c t) -> p c t", t=512)[:, 4 * hh: 4 * (hh + 1), :])
                else:
                    eng.dma_start(
                        out=xview[:, :, 512 * g: 512 * (g + 1)],
                        in_=x_d[g, :, :].rearrange("p (c t) -> p c t", t=512))

            # ---- constants ----
            ident_f = pp.tile([128, 128], f32, tag="idf")
            make_identity(nc, ident_f[:, :])
            ident_b = pp.tile([128, 128], bf16, tag="idb")
            nc.vector.tensor_copy(ident_b[:, :], ident_f[:, :])

            # mask_big[p, g] = 1 iff g >= p + 384 (else 0)
            mask_f = pp.tile([128, 896], f32, tag="mkf")
            nc.gpsimd.memset(mask_f[:, :], 0.0)
            nc.gpsimd.affine_select(
                out=mask_f[:, :], in_=mask_f[:, :],
                compare_op=mybir.AluOpType.is_gt, fill=1.0,
                base=384, pattern=[[-1, 896]], channel_multiplier=1,
            )
            mask_b = pp.tile([128, 896], bf16, tag="mkb")
            nc.vector.tensor_copy(mask_b[:, :], mask_f[:, :])

            # persistent activations
            kT = pp.tile([128, T], bf16, tag="kT")     # k^T; rows 64:128 dup
            qT = pp.tile([128, T], bf16, tag="qT")     # q^T; rows 0:64 dup
            # vsb chunk ch: cols 0:64 = v rows of key chunk ch, cols 64:128 = 1
            # -> PV matmul (m=128) yields numerator on psum rows 0:64 and the
            #    denominator replicated on rows 64:128 (same cycle count)
            vsb = pp.tile([128, 32 * 128], bf16, tag="vsb")
            nc.gpsimd.memset(vsb[:, :], 1.0)

            # preload the exp activation table early (hide the ~1.3us load)
            warm = pw.tile([1, 1], f32, tag="warm")
            nc.scalar.activation(warm[:, :], mask_b[0:1, 0:1], AF.Exp, scale=1.0)

            with (
                tc.tile_pool(name="psA", bufs=1, space="PSUM") as psA,
                tc.tile_pool(name="ps2", bufs=1, space="PSUM") as ps2,
                tc.tile_pool(name="ps3", bufs=1, space="PSUM") as ps3,
            ):
                # everything pre-allocated OUTSIDE the If (pool allocation
                # inside conditional branches breaks Tile's wait assignment)
                pjA = psA.tile([128, 512], f32, tag="pjA")   # [k|v] pass
                pjB = psA.tile([128, 512], f32, tag="pjB")   # vn transpose out
                ps_bufs = [ps2.tile([128, 1024], f32, tag=f"sc{i}", name=f"scb{i}") for i in range(2)]
                pT_bufs = [pw.tile([128, 1024], bf16, tag=f"pT{i}", name=f"pTb{i}") for i in range(3)]
                po_bufs = [ps3.tile([128, 512], f32, tag=f"po{i}", name=f"pob{i}") for i in range(2)]
                rc_bufs = [pw.tile([128, 512], f32, tag=f"rc{i}", name=f"rcb{i}") for i in range(2)]
                vT_bufs = [pw.tile([64, 512], bf16, tag=f"vT{i}", name=f"vTb{i}") for i in range(2)]
                outsb = pw.tile([64, 2048], f32, tag="outsb")

                def xrhs(dc, g):
                    return xsb[:, 4096 * dc + 512 * g: 4096 * dc + 512 * (g + 1)]

                def body(half):
                    qtiles = QTILES[half]
                    slot_of = {t: i for i, t in enumerate(qtiles)}
                    # pending epilogue pieces per po bank: the 3.4us DVE
                    # reciprocal would block the in-order DVE queue (and the
                    # kT dups the next scores need), so it is split into
                    # 128-col pieces dribbled between later pairs
                    state = {"gi": 0, "pend": {0: [], 1: []}}

                    def dribble(k):
                        for bank in (0, 1):
                            pend = state["pend"][bank]
                            while k > 0 and pend:
                                pend.pop(0)()
                                k -= 1

                    def drain(bank):
                        pend = state["pend"][bank]
                        while pend:
                            pend.pop(0)()

                    def emit_qp(t, t2=None):
                        # query projection m=64, lands in the scores psum
                        # rotation (consumes one gi slot); a second tile
                        # col-packs into psum rows 64:128 and runs
                        # concurrently on the PE array
                        gi = state["gi"]
                        state["gi"] = gi + 1
                        qp = ps_bufs[gi % 2]
                        tsl = slice(512 * t, 512 * (t + 1))
                        for dc in range(8):
                            nc.tensor.matmul(
                                qp[0:64, 0:512],
                                lhsT=wq[:, 64 * dc: 64 * (dc + 1)],
                                rhs=xrhs(dc, t), start=(dc == 0), stop=(dc == 7))
                            if t2 is not None:
                                nc.tensor.matmul(
                                    qp[64:128, 0:512],
                                    lhsT=wq[:, 64 * dc: 64 * (dc + 1)],
                                    rhs=xrhs(dc, t2), start=(dc == 0), stop=(dc == 7))
                        nc.scalar.copy(qT[0:64, tsl], qp[0:64, 0:512])
                        nc.vector.tensor_copy(qT[64:128, tsl], qp[0:64, 0:512])
                        if t2 is not None:
                            tsl2 = slice(512 * t2, 512 * (t2 + 1))
                            nc.vector.tensor_copy(qT[64:128, tsl2], qp[64:128, 0:512])
                            nc.scalar.copy(qT[0:64, tsl2], qp[64:128, 0:512])

                    def emit_kv(g):
                        tsl = slice(512 * g, 512 * (g + 1))
                        for dc in range(8):
                            nc.tensor.matmul(
                                pjA[:, :], lhsT=wkv[:, 128 * dc: 128 * (dc + 1)],
                                rhs=xrhs(dc, g), start=(dc == 0), stop=(dc == 7))
                        nc.scalar.copy(kT[0:64, tsl], pjA[0:64, :])
                        nc.vector.tensor_copy(kT[64:128, tsl], pjA[0:64, :])
                        vT = vT_bufs[g % 2]
                        nc.scalar.copy(vT[:, :], pjA[64:128, :])
                        for c in range(4):
                            nc.tensor.matmul(
                                pjB[:, 64 * c: 64 * (c + 1)],
                                lhsT=vT[0:64, 128 * c: 128 * (c + 1)],
                                rhs=ident_b[0:64, 0:64], start=True, stop=True)
                        nc.vector.tensor_copy(
                            vsb[:, 512 * g: 512 * (g + 1)].rearrange(
                                "p (c h) -> p c h", h=128)[:, :, 0:64],
                            pjB[:, 0:256].rearrange("p (c h) -> p c h", h=64))

                    def emit_event(ev):
                        if ev[0] == "qp":
                            emit_qp(ev[1])
                        elif ev[0] == "qp2":
                            emit_qp(ev[1], ev[2])
                        else:
                            emit_kv(ev[1])

                    def emit_attn_run(run, filler):
                        gi0 = state["gi"]
                        pairs = []
                        for _, t, plo, phi in run:
                            if plo == 0:  # first PVs on this po bank: the
                                # prior user's epilogue must be fully emitted
                                drain(slot_of[t] % 2)
                            for p in range(plo, phi, 2):
                                pairs.append((t, slot_of[t], p, 4 * t + 4))

                        def emit_scores(j):
                            t, slot, p, n = pairs[j]
                            tsl = slice(512 * t, 512 * (t + 1))
                            ps = ps_bufs[(gi0 + j) % 2]
                            nc.tensor.matmul(
                                ps[:, 0:512],
                                lhsT=kT[0:64, 128 * p: 128 * (p + 1)],
                                rhs=qT[0:64, tsl], start=True, stop=True)
                            nc.tensor.matmul(
                                ps[:, 512:1024],
                                lhsT=kT[64:128, 128 * (p + 1): 128 * (p + 2)],
                                rhs=qT[64:128, tsl], start=True, stop=True)

                        emit_scores(0)
                        for j, (t, slot, p, n) in enumerate(pairs):
                            ps = ps_bufs[(gi0 + j) % 2]
                            pT = pT_bufs[(gi0 + j) % 3]
                            po = po_bufs[slot % 2]
                            nc.scalar.activation(pT[:, :], ps[:, :], AF.Exp, scale=SCALE)
                            # keep PE busy during exp(j)
                            if j + 1 < len(pairs):
                                emit_scores(j + 1)
                            elif filler is not None:
                                emit_event(filler)
                            for jj in range(2):
                                pp_ = p + jj
                                if pp_ >= n - 4:  # diagonal chunk: causal mask
                                    delta = 128 * (pp_ - (n - 4))
                                    nc.vector.tensor_mul(
                                        pT[:, 512 * jj: 512 * (jj + 1)],
                                        pT[:, 512 * jj: 512 * (jj + 1)],
                                        mask_b[:, 384 - delta: 896 - delta])
                                nc.tensor.matmul(
                                    po[:, :],
                                    lhsT=vsb[:, 128 * pp_: 128 * (pp_ + 1)],
                                    rhs=pT[:, 512 * jj: 512 * (jj + 1)],
                                    start=(pp_ == 0), stop=(pp_ == n - 1))
                            if p + 2 == n:
                                # epilogue pieces: shifted reciprocal of the
                                # replicated denominator + aligned multiply
                                rc = rc_bufs[slot % 2]

                                def piece(q, slot=slot, po=po, rc=rc):
                                    cs = slice(128 * q, 128 * (q + 1))
                                    os = slice(512 * slot + 128 * q,
                                               512 * slot + 128 * (q + 1))
                                    nc.vector.reciprocal(
                                        rc[0:64, cs], po[64:128, cs])
                                    nc.vector.tensor_mul(
                                        outsb[:, os], po[0:64, cs], rc[0:64, cs])

                                state["pend"][slot % 2].extend(
                                    (lambda q=q: piece(q)) for q in range(4))
                            else:
                                dribble(1)
                        state["gi"] = gi0 + len(pairs)

                    events = SCHED[half]
                    i = 0
                    while i < len(events):
                        if events[i][0] != "A":
                            emit_event(events[i])
                            dribble(1)
                            i += 1
                            continue
                        run = []
                        while i < len(events) and events[i][0] == "A":
                            run.append(events[i])
                            i += 1
                        filler = None
                        if i < len(events):
                            filler = events[i]
                            i += 1
                        emit_attn_run(run, filler)
                    drain(0)
                    drain(1)

                pid = nc.partition_id(engines=[
                    mybir.EngineType.PE, mybir.EngineType.Activation,
                    mybir.EngineType.DVE])
                with tc.If(pid < 4) as cmp:
                    body(0)
                with cmp.Else():
                    body(1)

                # store after the If, sliced per slot so early slots overlap;
                # the last slot goes out in quarters as its epilogue pieces
                # complete
                for slot in range(3):
                    nc.sync.dma_start(
                        out=out_d[:, 512 * slot: 512 * (slot + 1)],
                        in_=outsb[:, 512 * slot: 512 * (slot + 1)])
                for q in range(4):
                    qs = slice(512 * 3 + 128 * q, 512 * 3 + 128 * (q + 1))
                    nc.sync.dma_start(out=out_d[:, qs], in_=outsb[:, qs])

    nc.compile()
    _CACHE["nc"] = nc
    return nc


def _in_maps(x, Wq, Wk, Wv):
    def pack128(w):  # [1024, m] -> [128, 8*m] partition layout
        m = w.shape[1]
        return np.ascontiguousarray(
            w.astype(BF16).reshape(8, 128, m).transpose(1, 0, 2).reshape(128, 8 * m))

    Wk, Wq, Wv = (np.asarray(a) for a in (Wk, Wq, Wv))
    wq_sb = pack128(Wq)
    wkv_sb = pack128(np.concatenate([Wk, Wv], axis=1))
    # X5[g, p, c, t'] = x[512g+t', 128c+p]  (8KB contiguous per (g, p))
    xts = [np.ascontiguousarray(
        np.asarray(x[b]).astype(BF16).reshape(8, 512, 8, 128)
        .transpose(0, 3, 2, 1).reshape(8, 128, 4096)) for b in range(B)]
    maps = []
    for c in range(NCORES):
        b = c % 4
        maps.append({"xt": xts[b], "wq": wq_sb, "wkv": wkv_sb})
    return maps


def _install_profile_shim():
    import sys, types
    import concourse.bass_utils as bu
    bu.upload_artifacts = lambda tmpdir: "local://" + tmpdir
    if "antenv.axon_hooks" in sys.modules:
        return
    mod = types.ModuleType("antenv.axon_hooks")
    holder = []
    mod.set_axon_ntff_profile_hook = holder.append
    mod.get_axon_ntff_profile_hook = lambda: holder[-1] if holder else None
    sys.modules["antenv.axon_hooks"] = mod
    import antenv
    antenv.axon_hooks = mod
    from trn_agent_boot.trn_boot import _ntff_profile_via_ctypes
    mod.set_axon_ntff_profile_hook(_ntff_profile_via_ctypes("/opt/axon/libaxon_pjrt.so"))


def kernel(x, Wq, Wk, Wv, _want_profile=False):
    if _want_profile:
        _install_profile_shim()
    nc = _build()
    maps = _in_maps(x, Wq, Wk, Wv)
    res = run_bass_kernel_spmd(nc, maps, core_ids=list(range(NCORES)),
                               trace=_want_profile)
    out = np.empty((B, T, H), np.float32)
    for c in range(NCORES):
        b, half = c % 4, c // 4
        r = np.asarray(res.results[c]["out"])  # [64, 2048]
        for slot, t in enumerate(QTILES[half]):
            out[b, 512 * t: 512 * (t + 1)] = r[:, 512 * slot: 512 * (slot + 1)].T
    if _want_profile:
        return out, res
    return out


# revision 58
# speedup vs baseline: 1.0812x; 1.0379x over previous
"""Causal single-head attention (B=4, T=4096, D=1024, H=64) on 8 TRN2 cores.

Sharding: 2 cores per batch; queries split for causal load balance:
  half0 (cores 0-3):  query tiles {0,3,4,7} (x512 rows), keys [0,4096)
  half1 (cores 4-7):  query tiles {1,2,5,6},              keys [0,3584)
Both halves own 72 key chunks of attention work.

The host pre-transposes and bf16-casts x per core (x^T, group-major so
every (group, partition) DMA line is one contiguous 8KB run).  Weights
are host-packed into SBUF partition layout: [Wq] and [Wk|Wv].
HBM traffic per core: 8.4MB bf16 instead of 16.8MB f32.

One fully-specialized If/Else (engines PE/Act/DVE); everything except
DMAs runs inside the branch.  Query projections (m=64 Wq pass) are
emitted early and independently of the causal key order, so each tile's
fill chunks run as soon as their keys project; only diagonal chunks
wait for their own group.  Attention segments are merged into pipelined
runs (scores of pair i+1 emitted during exp(i) on Act; the next
projection serves as PE filler under the last exp of a run).

Per key group g: pkv[128,512] = [Wk|Wv]^T x^T (single m=128 pass); the
kT row dup for row-packed score pairs comes from partition-shifted
Act/DVE copies straight out of PSUM.  PV accumulates vsb chunks
[v | ones] (m=128) so the softmax denominator lands replicated on psum
rows 64:128: epilogue = partition-shifted DVE reciprocal + aligned
multiply.  Output stays transposed [64, 2048]; the host transposes
back during unsharding.
"""

import numpy as np
import ml_dtypes

import concourse.bass as bass
import concourse.mybir as mybir
from concourse import bacc
from concourse.tile import TileContext
from concourse.masks import make_identity
from concourse.bass_utils import run_bass_kernel_spmd

B, T, D, H = 4, 4096, 1024, 64
NCORES = 8
NQ = 2048
SCALE = 1.0 / np.sqrt(D)  # 1/32
BF16 = ml_dtypes.bfloat16

QTILES = {0: [0, 3, 4, 7], 1: [1, 2, 5, 6]}
XORDER = [0, 1, 3, 2, 4, 5, 6, 7]

# event schedules: ("qp", tile) = query projection, ("kv", group) = key/value
# projection, ("A", tile, plo, phi) = attention pairs for chunk positions
# [plo, phi) of that tile (chunk c is ready once kv(c//4) ran; diagonal
# chunks 4t..4t+3 are in group t; tile t's n = 4t+4)
SCHED = {
    0: [("qp", 0), ("kv", 0), ("A", 0, 0, 4),
        ("qp", 3), ("kv", 1), ("A", 3, 0, 8),
        ("kv", 2), ("A", 3, 8, 12),
        ("kv", 3), ("A", 3, 12, 16),
        ("qp", 4), ("A", 4, 0, 16),
        ("kv", 4), ("A", 4, 16, 20),
        ("qp", 7), ("kv", 5), ("A", 7, 0, 24),
        ("kv", 6), ("A", 7, 24, 26),
        ("kv", 7), ("A", 7, 28, 32), ("A", 7, 26, 28)],
    1: [("kv", 0), ("qp", 1), ("kv", 1), ("A", 1, 0, 8),
        ("qp", 2), ("kv", 2), ("A", 2, 0, 12),
        ("qp", 5), ("kv", 3), ("A", 5, 0, 16),
        ("kv", 4), ("A", 5, 16, 20),
        ("qp", 6), ("kv", 5), ("A", 5, 20, 24), ("A", 6, 0, 22),
        ("kv", 6), ("A", 6, 24, 28), ("A", 6, 22, 24)],
}

_CACHE = {}


def _build():
    if "nc" in _CACHE:
        return _CACHE["nc"]
    f32 = mybir.dt.float32
    bf16 = mybir.dt.bfloat16
    AF = mybir.ActivationFunctionType

    nc = bacc.Bacc(None, target_bir_lowering=False)
    # x^T in group-major layout: x_d[g, p, c*512+t'] = x[512g+t', 128c+p]
    x_d = nc.declare_dram_parameter("xt", [8, 128, 4096], bf16, isOutput=False)
    wq_d = nc.declare_dram_parameter("wq", [128, 512], bf16, isOutput=False)
    wkv_d = nc.declare_dram_parameter("wkv", [128, 1024], bf16, isOutput=False)
    out_d = nc.declare_dram_parameter("out", [H, NQ], bf16, isOutput=True)

    with TileContext(nc) as tc:
        with (
            tc.tile_pool(name="persist", bufs=1) as pp,
            tc.tile_pool(name="work", bufs=2) as pw,
        ):
            # ---- x^T group 0 split across sync+gpsimd issue, then weights ----
            xsb = pp.tile([128, 8 * T], bf16, tag="xsb")  # [p, (dc, t)]
            xview = xsb[:, :].rearrange("p (c t) -> p c t", t=T)
            g0 = XORDER[0]
            for hh, eng in ((0, nc.sync), (1, nc.gpsimd)):
                eng.dma_start(
                    out=xview[:, 4 * hh: 4 * (hh + 1), 512 * g0: 512 * (g0 + 1)],
                    in_=x_d[g0, :, :].rearrange(
                        "p (c t) -> p c t", t=512)[:, 4 * hh: 4 * (hh + 1), :])
            wq = pp.tile([128, 512], bf16, tag="wq")
            nc.sync.dma_start(out=wq[:, :], in_=wq_d[:, :])
            wkv = pp.tile([128, 1024], bf16, tag="wkv")
            nc.gpsimd.dma_start(out=wkv[:, :], in_=wkv_d[:, :])
            for i, g in enumerate(XORDER[1:], start=1):
                eng = nc.gpsimd if i < 4 else nc.sync
                eng.dma_start(
                    out=xview[:, :, 512 * g: 512 * (g + 1)],
                    in_=x_d[g, :, :].rearrange("p (c t) -> p c t", t=512))

            # ---- constants ----
            ident_f = pp.tile([128, 128], f32, tag="idf")
            make_identity(nc, ident_f[:, :])
            ident_b = pp.tile([128, 128], bf16, tag="idb")
            nc.vector.tensor_copy(ident_b[:, :], ident_f[:, :])

            # mask_big[p, g] = 1 iff g >= p + 384 (else 0)
            mask_f = pp.tile([128, 896], f32, tag="mkf")
            nc.gpsimd.memset(mask_f[:, :], 0.0)
            nc.gpsimd.affine_select(
                out=mask_f[:, :], in_=mask_f[:, :],
                compare_op=mybir.AluOpType.is_gt, fill=1.0,
                base=384, pattern=[[-1, 896]], channel_multiplier=1,
            )
            mask_b = pp.tile([128, 896], bf16, tag="mkb")
            nc.vector.tensor_copy(mask_b[:, :], mask_f[:, :])

            # persistent activations
            kT = pp.tile([128, T], bf16, tag="kT")     # k^T; rows 64:128 dup
            qT = pp.tile([128, T], bf16, tag="qT")     # q^T; rows 0:64 dup
            # vsb chunk ch: cols 0:64 = v rows of key chunk ch, cols 64:128 = 1
            # -> PV matmul (m=128) yields numerator on psum rows 0:64 and the
            #    denominator replicated on rows 64:128 (same cycle count)
            vsb = pp.tile([128, 32 * 128], bf16, tag="vsb")
            nc.gpsimd.memset(vsb[:, :], 1.0)

            # preload the exp activation table early (hide the ~1.3us load)
            warm = pw.tile([1, 1], f32, tag="warm")
            nc.scalar.activation(warm[:, :], mask_b[0:1, 0:1], AF.Exp, scale=1.0)

            with (
                tc.tile_pool(name="psA", bufs=1, space="PSUM") as psA,
                tc.tile_pool(name="ps2", bufs=1, space="PSUM") as ps2,
                tc.tile_pool(name="ps3", bufs=1, space="PSUM") as ps3,
            ):
                # everything pre-allocated OUTSIDE the If (pool allocation
                # inside conditional branches breaks Tile's wait assignment)
                pjA = psA.tile([128, 512], f32, tag="pjA")   # [k|v] pass
                pjB = psA.tile([128, 512], f32, tag="pjB")   # vn transpose out
                ps_bufs = [ps2.tile([128, 1024], f32, tag=f"sc{i}", name=f"scb{i}") for i in range(2)]
                pT_bufs = [pw.tile([128, 1024], bf16, tag=f"pT{i}", name=f"pTb{i}") for i in range(3)]
                po_bufs = [ps3.tile([128, 512], f32, tag=f"po{i}", name=f"pob{i}") for i in range(2)]
                rc_bufs = [pw.tile([128, 512], f32, tag=f"rc{i}", name=f"rcb{i}") for i in range(2)]
                vT_bufs = [pw.tile([64, 512], bf16, tag=f"vT{i}", name=f"vTb{i}") for i in range(2)]
                outsb = pw.tile([64, 2048], bf16, tag="outsb")

                def xrhs(dc, g):
                    return xsb[:, 4096 * dc + 512 * g: 4096 * dc + 512 * (g + 1)]

                def body(half):
                    qtiles = QTILES[half]
                    slot_of = {t: i for i, t in enumerate(qtiles)}
                    # pending epilogue pieces per po bank: the 3.4us DVE
                    # reciprocal would block the in-order DVE queue (and the
                    # kT dups the next scores need), so it is split into
                    # 128-col pieces dribbled between later pairs
                    state = {"gi": 0, "pend": {0: [], 1: []}, "cnt": {}}

                    def dribble(k):
                        for bank in (0, 1):
                            pend = state["pend"][bank]
                            while k > 0 and pend:
                                pend.pop(0)()
                                k -= 1

                    def drain(bank):
                        pend = state["pend"][bank]
                        while pend:
                            pend.pop(0)()

                    def emit_qp(t, t2=None):
                        # query projection m=64, lands in the scores psum
                        # rotation (consumes one gi slot); a second tile
                        # col-packs into psum rows 64:128 and runs
                        # concurrently on the PE array
                        gi = state["gi"]
                        state["gi"] = gi + 1
                        qp = ps_bufs[gi % 2]
                        tsl = slice(512 * t, 512 * (t + 1))
                        for dc in range(8):
                            nc.tensor.matmul(
                                qp[0:64, 0:512],
                                lhsT=wq[:, 64 * dc: 64 * (dc + 1)],
                                rhs=xrhs(dc, t), start=(dc == 0), stop=(dc == 7))
                            if t2 is not None:
                                nc.tensor.matmul(
                                    qp[64:128, 0:512],
                                    lhsT=wq[:, 64 * dc: 64 * (dc + 1)],
                                    rhs=xrhs(dc, t2), start=(dc == 0), stop=(dc == 7))
                        nc.scalar.copy(qT[0:64, tsl], qp[0:64, 0:512])
                        nc.vector.tensor_copy(qT[64:128, tsl], qp[0:64, 0:512])
                        if t2 is not None:
                            tsl2 = slice(512 * t2, 512 * (t2 + 1))
                            nc.vector.tensor_copy(qT[64:128, tsl2], qp[64:128, 0:512])
                            nc.scalar.copy(qT[0:64, tsl2], qp[64:128, 0:512])

                    def emit_kv(g):
                        tsl = slice(512 * g, 512 * (g + 1))
                        for dc in range(8):
                            nc.tensor.matmul(
                                pjA[:, :], lhsT=wkv[:, 128 * dc: 128 * (dc + 1)],
                                rhs=xrhs(dc, g), start=(dc == 0), stop=(dc == 7))
                        nc.scalar.copy(kT[0:64, tsl], pjA[0:64, :])
                        nc.vector.tensor_copy(kT[64:128, tsl], pjA[0:64, :])
                        vT = vT_bufs[g % 2]
                        nc.scalar.copy(vT[:, :], pjA[64:128, :])
                        for c in range(4):
                            nc.tensor.matmul(
                                pjB[:, 64 * c: 64 * (c + 1)],
                                lhsT=vT[0:64, 128 * c: 128 * (c + 1)],
                                rhs=ident_b[0:64, 0:64], start=True, stop=True)
                        nc.vector.tensor_copy(
                            vsb[:, 512 * g: 512 * (g + 1)].rearrange(
                                "p (c h) -> p c h", h=128)[:, :, 0:64],
                            pjB[:, 0:256].rearrange("p (c h) -> p c h", h=64))

                    def emit_event(ev):
                        if ev[0] == "qp":
                            emit_qp(ev[1])
                        elif ev[0] == "qp2":
                            emit_qp(ev[1], ev[2])
                        else:
                            emit_kv(ev[1])

                    def emit_attn_run(run, filler):
                        gi0 = state["gi"]
                        pairs = []
                        for _, t, plo, phi in run:
                            if plo == 0:  # first PVs on this po bank: the
                                # prior user's epilogue must be fully emitted
                                drain(slot_of[t] % 2)
                            for p in range(plo, phi, 2):
                                pairs.append((t, slot_of[t], p, 4 * t + 4))

                        def emit_scores(j):
                            t, slot, p, n = pairs[j]
                            tsl = slice(512 * t, 512 * (t + 1))
                            ps = ps_bufs[(gi0 + j) % 2]
                            nc.tensor.matmul(
                                ps[:, 0:512],
                                lhsT=kT[0:64, 128 * p: 128 * (p + 1)],
                                rhs=qT[0:64, tsl], start=True, stop=True)
                            nc.tensor.matmul(
                                ps[:, 512:1024],
                                lhsT=kT[64:128, 128 * (p + 1): 128 * (p + 2)],
                                rhs=qT[64:128, tsl], start=True, stop=True)

                        emit_scores(0)
                        for j, (t, slot, p, n) in enumerate(pairs):
                            ps = ps_bufs[(gi0 + j) % 2]
                            pT = pT_bufs[(gi0 + j) % 3]
                            po = po_bufs[slot % 2]
                            nc.scalar.activation(pT[:, :], ps[:, :], AF.Exp, scale=SCALE)
                            # keep PE busy during exp(j)
                            if j + 1 < len(pairs):
                                emit_scores(j + 1)
                            elif filler is not None:
                                emit_event(filler)
                            for jj in range(2):
                                pp_ = p + jj
                                if pp_ >= n - 4:  # diagonal chunk: causal mask
                                    delta = 128 * (pp_ - (n - 4))
                                    nc.vector.tensor_mul(
                                        pT[:, 512 * jj: 512 * (jj + 1)],
                                        pT[:, 512 * jj: 512 * (jj + 1)],
                                        mask_b[:, 384 - delta: 896 - delta])
                                cnt = state["cnt"].get(t, 0)
                                state["cnt"][t] = cnt + 1
                                nc.tensor.matmul(
                                    po[:, :],
                                    lhsT=vsb[:, 128 * pp_: 128 * (pp_ + 1)],
                                    rhs=pT[:, 512 * jj: 512 * (jj + 1)],
                                    start=(cnt == 0), stop=(cnt == n - 1))
                            if state["cnt"][t] == n:
                                # epilogue pieces: shifted reciprocal of the
                                # replicated denominator + aligned multiply.
                                # slot 3 drains at the very end with nothing
                                # to dribble into, so use fewer, wider pieces
                                rc = rc_bufs[slot % 2]
                                npc, w = (2, 256) if slot == 3 else (4, 128)

                                def piece(q, slot=slot, po=po, rc=rc, w=w):
                                    cs = slice(w * q, w * (q + 1))
                                    os = slice(512 * slot + w * q,
                                               512 * slot + w * (q + 1))
                                    nc.vector.reciprocal(
                                        rc[0:64, cs], po[64:128, cs])
                                    nc.vector.tensor_mul(
                                        outsb[:, os], po[0:64, cs], rc[0:64, cs])

                                state["pend"][slot % 2].extend(
                                    (lambda q=q: piece(q)) for q in range(npc))
                            else:
                                dribble(1)
                        state["gi"] = gi0 + len(pairs)

                    events = SCHED[half]
                    i = 0
                    while i < len(events):
                        if events[i][0] != "A":
                            emit_event(events[i])
                            dribble(1)
                            i += 1
                            continue
                        run = []
                        while i < len(events) and events[i][0] == "A":
                            run.append(events[i])
                            i += 1
                        filler = None
                        if i < len(events):
                            filler = events[i]
                            i += 1
                        emit_attn_run(run, filler)
                    drain(0)
                    drain(1)

                pid = nc.partition_id(engines=[
                    mybir.EngineType.PE, mybir.EngineType.Activation,
                    mybir.EngineType.DVE])
                with tc.If(pid < 4) as cmp:
                    body(0)
                with cmp.Else():
                    body(1)

                # store after the If, sliced per slot so early slots overlap;
                # the last slot goes out in quarters as its epilogue pieces
                # complete
                for slot in range(3):
                    nc.sync.dma_start(
                        out=out_d[:, 512 * slot: 512 * (slot + 1)],
                        in_=outsb[:, 512 * slot: 512 * (slot + 1)])
                for q in range(2):
                    qs = slice(512 * 3 + 256 * q, 512 * 3 + 256 * (q + 1))
                    nc.sync.dma_start(out=out_d[:, qs], in_=outsb[:, qs])

    nc.compile()
    _CACHE["nc"] = nc
    return nc


def _in_maps(x, Wq, Wk, Wv):
    def pack128(w):  # [1024, m] -> [128, 8*m] partition layout
        m = w.shape[1]
        return np.ascontiguousarray(
            w.astype(BF16).reshape(8, 128, m).transpose(1, 0, 2).reshape(128, 8 * m))

    Wk, Wq, Wv = (np.asarray(a) for a in (Wk, Wq, Wv))
    wq_sb = pack128(Wq)
    wkv_sb = pack128(np.concatenate([Wk, Wv], axis=1))
    # X5[g, p, c, t'] = x[512g+t', 128c+p]  (8KB contiguous per (g, p))
    xts = [np.ascontiguousarray(
        np.asarray(x[b]).astype(BF16).reshape(8, 512, 8, 128)
        .transpose(0, 3, 2, 1).reshape(8, 128, 4096)) for b in range(B)]
    maps = []
    for c in range(NCORES):
        b = c % 4
        maps.append({"xt": xts[b], "wq": wq_sb, "wkv": wkv_sb})
    return maps


def _install_profile_shim():
    import sys, types
    import concourse.bass_utils as bu
    bu.upload_artifacts = lambda tmpdir: "local://" + tmpdir
    if "antenv.axon_hooks" in sys.modules:
        return
    mod = types.ModuleType("antenv.axon_hooks")
    holder = []
    mod.set_axon_ntff_profile_hook = holder.append
    mod.get_axon_ntff_profile_hook = lambda: holder[-1] if holder else None
    sys.modules["antenv.axon_hooks"] = mod
    import antenv
    antenv.axon_hooks = mod
    from trn_agent_boot.trn_boot import _ntff_profile_via_ctypes
    mod.set_axon_ntff_profile_hook(_ntff_profile_via_ctypes("/opt/axon/libaxon_pjrt.so"))


def kernel(x, Wq, Wk, Wv, _want_profile=False):
    if _want_profile:
        _install_profile_shim()
    nc = _build()
    maps = _in_maps(x, Wq, Wk, Wv)
    res = run_bass_kernel_spmd(nc, maps, core_ids=list(range(NCORES)),
                               trace=_want_profile)
    out = np.empty((B, T, H), np.float32)
    for c in range(NCORES):
        b, half = c % 4, c // 4
        r = np.asarray(res.results[c]["out"]).astype(np.float32)  # [64, 2048]
        for slot, t in enumerate(QTILES[half]):
            out[b, 512 * t: 512 * (t + 1)] = r[:, 512 * slot: 512 * (slot + 1)].T
    if _want_profile:
        return out, res
    return out
